# revision 63
# baseline (speedup 1.0000x reference)
"""Trainium2 Bass kernel for DenseGatPerfPlayerModel (2-layer masked GAT + MLP head).

Strategy (8 NeuronCores, data-parallel over batch B=32 -> G=4 graphs/core):
  - All matmul operands in bf16: fp32 matmuls run the PE in LOW_HIGH double-pass
    mode (~1.1us per 512-col matmul vs ~0.25us bf16) and disable fast weight
    load. Adjacency/masks/features are exact or near-exact in bf16.
  - Per-graph tensors feature-major ([feat, node]); scores computed in
    [m(source), n(dest)] layout, softmax over m; exp without max-subtraction
    (scores O(1)); denominator via ones-column in v; mask applied as
    pm = exp(sT) * adjT (bf16).
  - PE quadrant rule: K<=32 operands at partition base {0,32,64}; heads at
    32-partition stride, groups of (3,3,2).
  - Emission is phase-interleaved across graphs so every engine's in-order
    queue stays full: A(g)=load+proj, B(g)=attention inner loop (software
    pipelined: score-mms of iter i+1 are emitted before o-mms of iter i),
    C(g)=softmax finish + Wl + elu, D(g)=layer 2 at the query node only.
    Round order: A0, [B(g), D(g-1), C(g), A(g+1)] for g=0..3, D3, MLP.
  - Engine balance: exp on ACT (bottleneck, ~1.15us/tile), mask-mult on
    GpSimd (SBUF-only operands), PSUM->SBUF copies on DVE, relu-part of elu
    on DVE (max), single weight-blob DMA.

Host-side work is data marshaling only: shard over cores, device layouts,
bf16 casts, bias folding (ones-row augmentation), one-hot/query-row vectors.
"""

import numpy as np
import ml_dtypes

B, N = 32, 512
G = 4  # graphs per core
NCORES = 8
H, DH, DO, DLIN = 8, 16, 16, 64
DIN, DINIT = 16, 64
SCALE = 1999853.335557038
P = 128
MC = N // P  # 4 m-chunks per graph
NG = 3
GSZ = [3, 3, 2]
HMAP = [(h // 3, h % 3) for h in range(H)]
PAIRS = ((0, 3), (1, 4), (2, 6), (5, 7))  # cross-group head pairs

BF = ml_dtypes.bfloat16


def _blob_layout():
    """Column layout of the bf16 weight blob (all tensors at row 0)."""
    L = {}
    c = 0

    def add(name, rows, cols):
        nonlocal c
        L[name] = (rows, c, cols)
        c += cols

    add("Wi", DIN + 1, DINIT)
    for grp in range(NG):
        add(f"Wq0_{grp}", 65, P)
        add(f"Wk0_{grp}", 65, P)
    add("Wv0", 65, H * DO)
    add("Wq1c", 65, P)
    add("Wk1c", 65, P)
    add("Wv1", 65, H * DO)
    for grp in range(NG):
        add(f"Wl0_{grp}", 32 * GSZ[grp], DLIN)
        add(f"E{grp}", H, P)
        add(f"Sel{grp}", 32 * GSZ[grp], H)
    # layer-2 compact (h,32e) layout: halves of 4 heads at 32-stride
    for hf in range(2):
        add(f"Sel2c{hf}", P, H)
        add(f"E2c{hf}", H, P)
        add(f"Wl1c{hf}", P, DLIN)
    add("I64p", DLIN + 1, DLIN)
    add("QMaskC", P, H)  # block-diag 0/1: rows 16h..16h+16 of col h
    return L, c


def _f32_layout():
    F = {}
    c = 0

    def add(name, rows, cols):
        nonlocal c
        F[name] = (rows, c, cols)
        c += cols

    add("Wf0", 2 * DLIN, 128)
    add("Wf1", 128, 64)
    add("Wf2", 64, 1)
    add("bl0", DLIN, 1)
    add("bl1", DLIN, 1)
    add("bf0", 128, 1)
    add("bf1", 64, 1)
    add("bf2", 1, 1)
    return F, c


BLOB_L, WCOLS = _blob_layout()
F32_L, FCOLS = _f32_layout()


def _build_nc():
    from contextlib import ExitStack

    import concourse.mybir as mybir
    import concourse.tile as tile
    from concourse import bacc

    f32 = mybir.dt.float32
    bf = mybir.dt.bfloat16
    AF = mybir.ActivationFunctionType
    ALU = mybir.AluOpType

    nc = bacc.Bacc()

    nf_d = nc.declare_dram_parameter("nf", [DIN + 1, G, N], bf, isOutput=False)
    adj_d = nc.declare_dram_parameter("adjT", [G, P, MC, N], bf, isOutput=False)
    aqoh_d = nc.declare_dram_parameter("aqoh", [P, G, 2, MC], bf, isOutput=False)
    aq32_d = nc.declare_dram_parameter("aq32", [P, G, MC], f32, isOutput=False)
    wb_d = nc.declare_dram_parameter("wb", [P, WCOLS], bf, isOutput=False)
    wf_d = nc.declare_dram_parameter("wf", [P, FCOLS], f32, isOutput=False)
    out_d = nc.declare_dram_parameter("out", [1, G], f32, isOutput=True)

    with tile.TileContext(nc) as tc, ExitStack() as ctx:
        wpool = ctx.enter_context(tc.tile_pool(name="w", bufs=1))
        gpool = ctx.enter_context(tc.tile_pool(name="g", bufs=2))
        work = ctx.enter_context(tc.tile_pool(name="work", bufs=3))
        misc = ctx.enter_context(tc.tile_pool(name="misc", bufs=2))
        persist = ctx.enter_context(tc.tile_pool(name="persist", bufs=1))
        ps_s = ctx.enter_context(tc.tile_pool(name="ps_s", bufs=2, space="PSUM"))
        ps_o = ctx.enter_context(tc.tile_pool(name="ps_o", bufs=1, space="PSUM"))
        ps_m = ctx.enter_context(tc.tile_pool(name="ps_m", bufs=1, space="PSUM"))

        # ---- DMAs (all issued up front; transfers overlap compute).
        # Order matters for the head: graph 0's critical inputs first.
        wb_sb = wpool.tile([P, WCOLS], bf)
        nfT = wpool.tile([DIN + 1, G, N], bf)
        adjb = wpool.tile([P, G, MC, N], bf)
        wf_sb = wpool.tile([P, FCOLS], f32)
        aqoh = wpool.tile([P, G, 2, MC], bf)
        aq32 = wpool.tile([P, G, MC], f32)
        WB1 = 960  # Wi + layer-1 q/k/v weights: everything graph 0's A needs
        nc.sync.dma_start(wb_sb[:, 0:WB1], wb_d[:, 0:WB1])
        nc.sync.dma_start(nfT[:], nf_d[:])
        nc.sync.dma_start(adjb[:, 0, :, :], adj_d[0])
        nc.sync.dma_start(wb_sb[:, WB1:], wb_d[:, WB1:])
        nc.sync.dma_start(aqoh[:], aqoh_d[:])
        nc.sync.dma_start(aq32[:], aq32_d[:])
        nc.sync.dma_start(wf_sb[:], wf_d[:])
        for g_ in range(1, G):
            nc.sync.dma_start(adjb[:, g_, :, :], adj_d[g_])

        W = {k: wb_sb[0:r, c:c + n] for k, (r, c, n) in BLOB_L.items()}
        F = {k: wf_sb[0:r, c:c + n] for k, (r, c, n) in F32_L.items()}
        ones1 = wpool.tile([1, 1], bf)
        nc.vector.memset(ones1[:], 1.0)
        ones128 = wpool.tile([P, 1], bf)
        nc.vector.memset(ones128[:], 1.0)
        # prime the ACT exp table while input DMAs stream
        warm = wpool.tile([1, 1], f32)
        nc.scalar.activation(warm[:], ones1[:], AF.Exp)

        feat_sb = persist.tile([P, G], f32)
        out_sb = persist.tile([1, G], f32)

        def elu(dst, src_ps, bias, p, f, dt, tg):
            # dst = elu(src+bias) = min(exp(src+bias)-1, 0) + max(src+bias, 0)
            # ACT does only the exp pass (it is the global bottleneck).
            e = work.tile([p, f], dt, tag=f"elu_e_{tg}", name=f"elu_e_{tg}")
            nc.scalar.activation(e[:], src_ps, AF.Exp, bias=bias)
            nc.vector.tensor_scalar(e[:], e[:], 1.0, 0.0, ALU.subtract, ALU.min)
            r = work.tile([p, f], dt, tag=f"elu_r_{tg}", name=f"elu_r_{tg}")
            if isinstance(bias, float):
                nc.vector.tensor_scalar(r[:], src_ps, bias, 0.0, ALU.add, ALU.max)
            else:
                nc.vector.tensor_scalar(r[:], src_ps, bias, 0.0, ALU.add, ALU.max)
            nc.vector.tensor_add(dst, e[:], r[:])

        # per-graph state
        x0a = [None] * G
        qkt = [[None] * NG for _ in range(G)]
        vsb = [None] * G
        o_ps_g = [None] * G
        x1a = [None] * G

        def chunksA(g):
            def c_x0():
                x0ps = ps_m.tile([DINIT, N], f32, tag="m", name="x0ps")
                nc.tensor.matmul(x0ps[:], W["Wi"][:], nfT[:, g, :],
                                 start=True, stop=True)
                xa = gpool.tile([DINIT + 1, N], bf, tag="x0a", name="x0a")
                elu(xa[0:DINIT, :], x0ps[:], 0.0, DINIT, N, bf, "x0")
                nc.vector.memset(xa[DINIT:DINIT + 1, :], 1.0)
                x0a[g] = xa

            def c_qk(grp):
                xa = x0a[g]
                sp = ps_s.tile([P, 2, N], f32, tag="s", name=f"qkp{grp}")
                nc.tensor.matmul(sp[:, 0, :], W[f"Wq0_{grp}"][:], xa[:],
                                 start=True, stop=True)
                nc.tensor.matmul(sp[:, 1, :], W[f"Wk0_{grp}"][:], xa[:],
                                 start=True, stop=True)
                qk = gpool.tile([P, 2, N], bf, tag=f"qk{grp}", name=f"qk{grp}")
                if grp == 1:
                    nc.scalar.copy(qk[:], sp[:])
                else:
                    nc.vector.tensor_copy(qk[:], sp[:])
                qkt[g][grp] = qk

            def c_v():
                xa = x0a[g]
                vps = ps_s.tile([P, MC, P], f32, tag="s", name="vps")
                for mc in range(MC):
                    nc.tensor.matmul(vps[:, mc, :], xa[:, mc * P:(mc + 1) * P],
                                     W["Wv0"][:], start=True, stop=True)
                vt = gpool.tile([P, MC, H, 32], bf, tag="vsb", name="vsb")
                nc.gpsimd.memset(vt[:], 0.0)
                nc.vector.memset(vt[:, :, :, DO:DO + 1], 1.0)
                for mc in range(MC):
                    nc.vector.tensor_copy(
                        vt[:, mc, :, 0:DO],
                        vps[:, mc, :].rearrange("p (h e) -> p h e", h=H))
                vsb[g] = vt

            return [c_x0, lambda: c_qk(0), lambda: c_qk(1), lambda: c_qk(2), c_v]

        def phaseB(g, stuffers=None):
            stuffers = stuffers or []
            o_ps = [ps_o.tile([P, N], f32, tag=f"o{grp}", name=f"o{grp}")
                    for grp in range(NG)]
            o_ps_g[g] = o_ps
            iters = [(pr, mc) for pr in PAIRS for mc in range(MC)]
            pend = []
            DEPTH = 1  # o-mms trail score-mms by 1 iteration
            for idx in range(len(iters) + DEPTH):
                if idx < len(iters):
                    (ha, hb), mc = iters[idx]
                    sp = ps_s.tile([P, 2, N], f32, tag="s", name="sp")
                    for r, h in enumerate((ha, hb)):
                        grp, pos = HMAP[h]
                        nc.tensor.matmul(
                            sp[:, r, :],
                            qkt[g][grp][32 * pos:32 * pos + DH, 1, mc * P:(mc + 1) * P],
                            qkt[g][grp][32 * pos:32 * pos + DH, 0, :],
                            start=True, stop=True)
                    ex = work.tile([P, 2, N], bf, tag="ex")
                    nc.scalar.activation(ex[:], sp[:], AF.Exp)
                    pm = work.tile([P, 2, N], bf, tag="pm", bufs=4)
                    nc.vector.tensor_tensor(
                        pm[:], ex[:],
                        adjb[:, g, mc, None, :].to_broadcast((P, 2, N)),
                        ALU.mult)
                    pend.append(((ha, hb), mc, pm))
                if idx >= DEPTH:
                    (ha, hb), mc, pmp = pend.pop(0)
                    for r, h in enumerate((ha, hb)):
                        grp, pos = HMAP[h]
                        nc.tensor.matmul(
                            o_ps[grp][32 * pos:32 * pos + 32, :],
                            vsb[g][:, mc, h, :], pmp[:, r, :],
                            start=(mc == 0), stop=(mc == MC - 1))
                if stuffers:
                    stuffers.pop(0)()
            while stuffers:
                stuffers.pop(0)()

        def chunksC(g):
            osb = []
            scrs = []
            rec_box = []

            def c_osb():
                o_ps = o_ps_g[g]
                for grp in range(NG):
                    rg = 32 * GSZ[grp]
                    t = gpool.tile([rg, N], bf, tag=f"osb{grp}", name=f"osb{grp}")
                    if grp == 1:
                        nc.scalar.copy(t[:], o_ps[grp][0:rg, :])
                    else:
                        nc.vector.tensor_copy(t[:], o_ps[grp][0:rg, :])
                    osb.append(t)

            def c_den():
                den = ps_m.tile([H, N], f32, tag="m", name="den")
                for grp in range(NG):
                    nc.tensor.matmul(den[:], W[f"Sel{grp}"][:], osb[grp][:],
                                     start=(grp == 0), stop=(grp == NG - 1))
                recf = misc.tile([H, N], f32, tag="recf")
                nc.vector.reciprocal_approx_fast(recf[:], den[:])
                rec = misc.tile([H, N], bf, tag="rec")
                nc.vector.tensor_copy(rec[:], recf[:])
                rec_box.append(rec)

            def c_scr(grp):
                rg = 32 * GSZ[grp]
                dps = ps_s.tile([P, N], f32, tag="s", name="dps")
                nc.tensor.matmul(dps[:], W[f"E{grp}"][:], rec_box[0][:],
                                 start=True, stop=True)
                scr = misc.tile([rg, N], bf, tag=f"scr{grp}", name=f"scr{grp}")
                nc.vector.tensor_tensor(scr[:], osb[grp][:], dps[0:rg, :], ALU.mult)
                scrs.append(scr)

            def c_x1():
                x1ps = ps_m.tile([DLIN, N], f32, tag="m", name="x1ps")
                for grp in range(NG):
                    nc.tensor.matmul(x1ps[:], W[f"Wl0_{grp}"][:], scrs[grp][:],
                                     start=(grp == 0), stop=(grp == NG - 1))
                xa = gpool.tile([DLIN + 1, N], bf, tag="x1a", name="x1a")
                elu(xa[0:DLIN, :], x1ps[:], F["bl0"][:], DLIN, N, bf, "x1")
                nc.vector.memset(xa[DLIN:DLIN + 1, :], 1.0)
                x1a[g] = xa

            return [c_osb, c_den, lambda: c_scr(0), lambda: c_scr(1),
                    lambda: c_scr(2), c_x1]

        def chunksD(g):
            st = {}

            def c_gather():
                xa = x1a[g]
                ndp = ps_s.tile([P, MC, DLIN], f32, tag="s", name="ndp")
                for mc in range(MC):
                    nc.tensor.matmul(ndp[:, mc, :], xa[:, mc * P:(mc + 1) * P],
                                     W["I64p"][:], start=True, stop=True)
                x1nd = gpool.tile([P, MC, DLIN], bf, tag="x1nd", bufs=1, name="x1nd")
                nc.vector.tensor_copy(x1nd[:], ndp[:])
                x1qps = ps_m.tile([DLIN, 1], f32, tag="m", name="x1qps")
                for mc in range(MC):
                    nc.tensor.matmul(x1qps[:], x1nd[:, mc, :],
                                     aqoh[:, g, 1, mc:mc + 1],
                                     start=(mc == 0), stop=(mc == MC - 1))
                nc.vector.tensor_copy(feat_sb[0:DLIN, g:g + 1], x1qps[:])
                x1qa = gpool.tile([DLIN + 1, 1], bf, tag="x1qa", bufs=1, name="x1qa")
                nc.vector.tensor_copy(x1qa[0:DLIN, :], x1qps[:])
                nc.vector.memset(x1qa[DLIN:DLIN + 1, :], 1.0)
                st["x1qa"] = x1qa

            def c_q2():
                q2ps = ps_m.tile([P, 1], f32, tag="m", name="q2ps")
                nc.tensor.matmul(q2ps[:], W["Wq1c"][:], st["x1qa"][:],
                                 start=True, stop=True)
                qb = gpool.tile([P, H], bf, tag="q2bdc", bufs=1, name="q2bdc")
                nc.vector.tensor_tensor(
                    qb[:], q2ps[:].to_broadcast((P, H)),
                    W["QMaskC"][:], ALU.mult)
                st["q2bd"] = qb

            def c_k2():
                k2ps = ps_s.tile([P, N], f32, tag="s", name="k2ps")
                nc.tensor.matmul(k2ps[:], W["Wk1c"][:], x1a[g][:],
                                 start=True, stop=True)
                kb = gpool.tile([P, N], bf, tag="k2c", bufs=1, name="k2c")
                nc.scalar.copy(kb[:], k2ps[:])
                st["k2c"] = kb

            def c_v2():
                xa = x1a[g]
                v2ps = ps_s.tile([P, MC, P], f32, tag="s", name="v2ps")
                for mc in range(MC):
                    nc.tensor.matmul(v2ps[:, mc, :], xa[:, mc * P:(mc + 1) * P],
                                     W["Wv1"][:], start=True, stop=True)
                v2 = gpool.tile([P, MC, H, 32], bf, tag="v2sb", bufs=1, name="v2sb")
                nc.gpsimd.memset(v2[:], 0.0)
                nc.vector.memset(v2[:, :, :, DO:DO + 1], 1.0)
                for mc in range(MC):
                    nc.vector.tensor_copy(
                        v2[:, mc, :, 0:DO],
                        v2ps[:, mc, :].rearrange("p (h e) -> p h e", h=H))
                    # fold query-row adjacency mask (incl. ones col)
                    nc.vector.tensor_scalar_mul(
                        v2[:, mc, :, :], v2[:, mc, :, :], aq32[:, g, mc:mc + 1])
                st["v2"] = v2

            def c_att():
                # all four m-chunks: one exp over [P, MC, H]
                k2c, q2bd, v2 = st["k2c"], st["q2bd"], st["v2"]
                s2p = ps_s.tile([P, MC, H], f32, tag="s", name="s2p")
                for mc in range(MC):
                    nc.tensor.matmul(s2p[:, mc, :],
                                     k2c[:, mc * P:(mc + 1) * P], q2bd[:],
                                     start=True, stop=True)
                ex2 = misc.tile([P, MC, H], bf, tag="ex2")
                nc.scalar.activation(ex2[:], s2p[:], AF.Exp)
                # weighted v, compact (h, 32e) columns; summed over m later
                for mc in range(MC):
                    w2 = misc.tile([P, H, 32], bf, tag="w2", bufs=4, name="w2")
                    nc.vector.tensor_tensor(
                        w2[:], v2[:, mc, :, :],
                        ex2[:, mc, :, None].to_broadcast((P, H, 32)), ALU.mult)
                    st.setdefault("w2", []).append(w2)

            def c_fin():
                w2 = st["w2"]
                a01 = misc.tile([P, H, 32], bf, tag="a01")
                nc.vector.tensor_add(a01[:], w2[0][:], w2[1][:])
                a23 = misc.tile([P, H, 32], bf, tag="a23")
                nc.vector.tensor_add(a23[:], w2[2][:], w2[3][:])
                accf = misc.tile([P, H * 32], bf, tag="accf")
                nc.vector.tensor_add(
                    accf[:], a01.rearrange("p h e -> p (h e)"),
                    a23.rearrange("p h e -> p (h e)"))
                # partition-reduce via ones-vector matmul -> [1, 256] row
                redps = ps_m.tile([1, H * 32], f32, tag="m", name="redps")
                nc.tensor.matmul(redps[:], ones128[:], accf[:],
                                 start=True, stop=True)
                red = misc.tile([1, H * 32], bf, tag="red")
                nc.vector.tensor_copy(red[:], redps[:])
                # transpose [1,256] -> two [128,1] psum columns via K=1 matmuls
                o2c = ps_m.tile([P, 2], f32, tag="m", name="o2c")
                for hf in range(2):
                    nc.tensor.matmul(o2c[:, hf:hf + 1],
                                     red[0:1, P * hf:P * (hf + 1)], ones1[:],
                                     start=True, stop=True)
                o2cb = misc.tile([P, 2], bf, tag="o2cb")
                nc.vector.tensor_copy(o2cb[:], o2c[:])
                den2 = ps_m.tile([H, 1], f32, tag="m", name="den2")
                for hf in range(2):
                    nc.tensor.matmul(den2[:], W[f"Sel2c{hf}"][:],
                                     o2cb[:, hf:hf + 1],
                                     start=(hf == 0), stop=(hf == 1))
                rec2f = misc.tile([H, 1], f32, tag="rec2f")
                nc.vector.reciprocal_approx_fast(rec2f[:], den2[:])
                rec2 = misc.tile([H, 1], bf, tag="rec2")
                nc.vector.tensor_copy(rec2[:], rec2f[:])
                d2ps = ps_m.tile([P, 2], f32, tag="m", name="d2ps")
                for hf in range(2):
                    nc.tensor.matmul(d2ps[:, hf:hf + 1], W[f"E2c{hf}"][:], rec2[:],
                                     start=True, stop=True)
                scr2c = misc.tile([P, 2], bf, tag="scr2c")
                nc.vector.tensor_tensor(scr2c[:], o2cb[:], d2ps[:], ALU.mult)
                x2ps = ps_m.tile([DLIN, 1], f32, tag="m", name="x2ps")
                for hf in range(2):
                    nc.tensor.matmul(x2ps[:], W[f"Wl1c{hf}"][:],
                                     scr2c[:, hf:hf + 1],
                                     start=(hf == 0), stop=(hf == 1))
                elu(feat_sb[DLIN:2 * DLIN, g:g + 1], x2ps[:], F["bl1"][:],
                    DLIN, 1, f32, "x2")

            return [c_k2, c_v2, c_gather, c_q2, c_att, c_fin]

        # ---- emission schedule ----
        # B(g) hosts stuffed chunks: C(g-1), D(g-1), A(g+1); C/D of the last
        # graph run exposed at the tail.
        for c in chunksA(0):
            c()
        phaseB(0, chunksA(1))
        for g in range(1, G):
            stuff = chunksC(g - 1) + chunksD(g - 1)
            if g + 1 < G:
                stuff += chunksA(g + 1)
            phaseB(g, stuff)
        for c in chunksC(G - 1) + chunksD(G - 1):
            c()

        # ---- MLP head over all graphs (fp32, tiny) ----
        h1ps = ps_m.tile([128, G], f32, tag="m", name="h1ps")
        nc.tensor.matmul(h1ps[:], F["Wf0"][:], feat_sb[:], start=True, stop=True)
        h1 = persist.tile([128, G], f32, tag="h1")
        elu(h1[:], h1ps[:], F["bf0"][:], 128, G, f32, "m1")
        h2ps = ps_m.tile([64, G], f32, tag="m", name="h2ps")
        nc.tensor.matmul(h2ps[:], F["Wf1"][:], h1[:], start=True, stop=True)
        h2 = persist.tile([64, G], f32, tag="h2")
        elu(h2[:], h2ps[:], F["bf1"][:], 64, G, f32, "m2")
        h3ps = ps_m.tile([1, G], f32, tag="m", name="h3ps")
        nc.tensor.matmul(h3ps[:], F["Wf2"][:], h2[:], start=True, stop=True)
        elu(out_sb[:], h3ps[:], F["bf2"][:], 1, G, f32, "m3")
        nc.vector.tensor_scalar_mul(out_sb[:], out_sb[:], float(SCALE))
        nc.sync.dma_start(out_d[:], out_sb[:])

    nc.compile()
    return nc


def _prep_weights(inputs):
    f32 = np.float32
    vals = {}

    def aug(Wm, bv):
        return np.concatenate([np.asarray(Wm, f32).reshape(Wm.shape[0], -1),
                               np.asarray(bv, f32).reshape(1, -1)], axis=0)

    def pad3(Wa):  # [d, H*DH] -> NG x [d, 128] group-padded (32-stride)
        outs = []
        for grp in range(NG):
            Om = np.zeros((Wa.shape[0], P), f32)
            for pos in range(GSZ[grp]):
                h = 3 * grp + pos
                Om[:, 32 * pos:32 * pos + DH] = Wa[:, DH * h:DH * (h + 1)]
            outs.append(Om)
        return outs

    s = 1.0 / np.sqrt(DH)
    vals["Wi"] = aug(inputs["W_init"], inputs["b_init"])
    for grp, Om in enumerate(pad3(aug(inputs["Wq0"], inputs["bq0"]))):
        vals[f"Wq0_{grp}"] = Om
    for grp, Om in enumerate(pad3(aug(np.asarray(inputs["Wk0"], f32) * s,
                                      np.asarray(inputs["bk0"], f32) * s))):
        vals[f"Wk0_{grp}"] = Om
    vals["Wv0"] = aug(inputs["Wv0"], inputs["bv0"])
    vals["Wq1c"] = aug(inputs["Wq1"], inputs["bq1"])
    vals["Wk1c"] = aug(np.asarray(inputs["Wk1"], f32) * s,
                       np.asarray(inputs["bk1"], f32) * s)
    vals["Wv1"] = aug(inputs["Wv1"], inputs["bv1"])
    Wl0 = np.asarray(inputs["Wl0"], f32)  # [H*DO, DLIN]
    for grp in range(NG):
        Wlp = np.zeros((32 * GSZ[grp], DLIN), f32)
        for pos in range(GSZ[grp]):
            h = 3 * grp + pos
            Wlp[32 * pos:32 * pos + DO] = Wl0[DO * h:DO * (h + 1)]
        vals[f"Wl0_{grp}"] = Wlp
    Wl1 = np.asarray(inputs["Wl1"], f32)
    for hf in range(2):
        Sel2c = np.zeros((P, H), f32)
        E2c = np.zeros((H, P), f32)
        Wl1c = np.zeros((P, DLIN), f32)
        for hh in range(4):
            h = 4 * hf + hh
            Sel2c[32 * hh + DO, h] = 1.0
            E2c[h, 32 * hh:32 * hh + DO + 1] = 1.0
            Wl1c[32 * hh:32 * hh + DO] = Wl1[DO * h:DO * (h + 1)]
        vals[f"Sel2c{hf}"] = Sel2c
        vals[f"E2c{hf}"] = E2c
        vals[f"Wl1c{hf}"] = Wl1c
    for grp in range(NG):
        E = np.zeros((H, P), f32)
        Sel = np.zeros((32 * GSZ[grp], H), f32)
        for pos in range(GSZ[grp]):
            E[3 * grp + pos, 32 * pos:32 * pos + DO + 1] = 1.0
            Sel[32 * pos + DO, 3 * grp + pos] = 1.0
        vals[f"E{grp}"] = E
        vals[f"Sel{grp}"] = Sel
    vals["I64p"] = np.concatenate([np.eye(DLIN, dtype=f32),
                                   np.zeros((1, DLIN), f32)], axis=0)
    QMaskC = np.zeros((P, H), f32)
    for h in range(H):
        QMaskC[DH * h:DH * (h + 1), h] = 1.0
    vals["QMaskC"] = QMaskC

    wb = np.zeros((P, WCOLS), f32)
    for k, (r, c, n) in BLOB_L.items():
        wb[0:r, c:c + n] = vals[k]

    fvals = {
        "Wf0": np.asarray(inputs["Wf0"], f32),
        "Wf1": np.asarray(inputs["Wf1"], f32),
        "Wf2": np.asarray(inputs["Wf2"], f32),
        "bl0": np.asarray(inputs["bl0"], f32).reshape(DLIN, 1),
        "bl1": np.asarray(inputs["bl1"], f32).reshape(DLIN, 1),
        "bf0": np.asarray(inputs["bf0"], f32).reshape(128, 1),
        "bf1": np.asarray(inputs["bf1"], f32).reshape(64, 1),
        "bf2": np.asarray(inputs["bf2"], f32).reshape(1, 1),
    }
    wf = np.zeros((P, FCOLS), f32)
    for k, (r, c, n) in F32_L.items():
        wf[0:r, c:c + n] = fvals[k]
    return {"wb": wb.astype(BF), "wf": wf}


def _prep_core_inputs(inputs, core):
    """Marshal one core's shard (graphs core*G .. core*G+G-1) into device layout."""
    f32 = np.float32
    sl = slice(core * G, (core + 1) * G)
    nf = np.asarray(inputs["node_features"], f32)[sl]     # [G, N, DIN]
    adj = np.asarray(inputs["adj"], f32)[sl]              # [G, N, N]
    masks = np.asarray(inputs["masks"], f32)[sl]          # [G, N]
    qidx = np.asarray(inputs["query_idxs"])[sl]           # [G]

    nfT = np.ones((DIN + 1, G, N), f32)
    nfT[0:DIN] = np.transpose(nf, (2, 0, 1))

    adjm = ((np.transpose(adj, (0, 2, 1)) > 0) & (masks[:, :, None] > 0)).astype(f32)
    # [G, N(m), N(n)] -> [G, 128(p), MC, N]: row mc*128+p -> (p, mc)
    adjdev = adjm.reshape(G, MC, P, N).transpose(0, 2, 1, 3)

    aqoh = np.zeros((P, G, 2, MC), f32)
    for g in range(G):
        aq = ((adj[g, qidx[g]] > 0) & (masks[g] > 0)).astype(f32)
        aqoh[:, g, 0, :] = aq.reshape(MC, P).T
        oh = np.zeros(N, f32)
        oh[qidx[g]] = 1.0
        aqoh[:, g, 1, :] = oh.reshape(MC, P).T
    return {
        "nf": np.ascontiguousarray(nfT).astype(BF),
        "adjT": np.ascontiguousarray(adjdev).astype(BF),
        "aqoh": np.ascontiguousarray(aqoh).astype(BF),
        "aq32": np.ascontiguousarray(aqoh[:, :, 0, :]),
    }


def kernel(**inputs) -> np.ndarray:
    from concourse.bass_utils import run_bass_kernel_spmd

    nc = _build_nc()
    w = _prep_weights(inputs)
    in_maps = []
    for core in range(NCORES):
        m = _prep_core_inputs(inputs, core)
        m.update(w)
        in_maps.append(m)
    res = run_bass_kernel_spmd(nc, in_maps, list(range(NCORES)))
    out = np.concatenate([res.results[i]["out"][0] for i in range(NCORES)])
    return out.astype(np.float32).reshape(B, 1)
